# revision 4
# baseline (speedup 1.0000x reference)
"""Causal self-attention (B=2, T=2048, C=1024, H=16) on 8 TRN2 NeuronCores.

Sharding: data-parallel over batch (2 groups of 4 cores) x tensor-parallel
over heads (4 heads per core). Each core computes qkv for its 4 heads,
causal flash-style attention, and a partial output projection
(y_heads @ w_proj rows). Host sums the 4 partial projections per batch and
adds b_proj.

Per-core device pipeline (all matmuls bf16, fp32 accumulation):
  x [2048,1024] --cast-dma--> bf16 --xbar-transpose--> xT [C-part, T]
  qT/kT = w.T @ xT  (head-pair tiles [128, T], 2 heads stacked)
  v     = xT.T @ wv ([T-part, 4 heads x 64], +ones column for rowsums)
  S^T[k,q] = kT.T@qT per 128k x 512q block (causal blocks only,
             lower-triangle via restricted N + additive mask on diagonal)
  P = exp(S^T/32)  (ACT, PSUM->SBUF bf16; no max-subtraction: scores are O(1))
  y'^T[d,q] (+rowsum row) = v_aug.T @ P  (PSUM accum over k blocks)
  yT = y'^T * (1/rowsum) via rank-1 broadcast matmul + reciprocal_approx
  outT tile = yT.T @ wp  -> out [T-part, C] partial, DMA to DRAM
"""
import threading

import numpy as np

import concourse.bass as bass
import concourse.tile as tile
from concourse import bacc, mybir
from concourse.bass2jax import (
    _bass_exec_p,
    install_neuronx_cc_hook,
    partition_id_tensor,
)

N_CORES = 8
B, T, C, H = 2, 2048, 1024, 16
D = C // H            # 64
HL = 4                # heads per core
G = H // HL           # 4 head groups
SCALE = 1.0 / 32.0    # 1/sqrt(C)
F32 = mybir.dt.float32
BF16 = mybir.dt.bfloat16
NEG = -1e9


def build_nc():
    nc = bacc.Bacc("TRN2", target_bir_lowering=False, debug=False,
                   num_devices=N_CORES)
    x = nc.dram_tensor("x", [T, C], F32, kind="ExternalInput").ap()
    wq = nc.dram_tensor("wq", [C, 256], F32, kind="ExternalInput").ap()
    wk = nc.dram_tensor("wk", [C, 256], F32, kind="ExternalInput").ap()
    wv = nc.dram_tensor("wv", [C, 256], F32, kind="ExternalInput").ap()
    bq = nc.dram_tensor("bq", [128, 2], F32, kind="ExternalInput").ap()
    bk = nc.dram_tensor("bk", [128, 2], F32, kind="ExternalInput").ap()
    bv = nc.dram_tensor("bv", [128, 2], F32, kind="ExternalInput").ap()
    wp = nc.dram_tensor("wp", [256, C], F32, kind="ExternalInput").ap()
    mask = nc.dram_tensor("mask", [128, 128], F32, kind="ExternalInput").ap()
    out = nc.dram_tensor("out", [T, C], F32, kind="ExternalOutput").ap()

    Exp = mybir.ActivationFunctionType.Exp
    add = mybir.AluOpType.add
    mult = mybir.AluOpType.mult

    with tile.TileContext(nc) as tc:
        with tc.tile_pool(name="dram", bufs=1, space="DRAM") as dp, \
             tc.tile_pool(name="const", bufs=1) as cp, \
             tc.tile_pool(name="ps", bufs=3, space="PSUM") as psp, \
             tc.tile_pool(name="pvps", bufs=2, space="PSUM") as pvp, \
             tc.tile_pool(name="bcps", bufs=2, space="PSUM") as bcp, \
             tc.tile_pool(name="ptp", bufs=4) as ptp, \
             tc.tile_pool(name="rowp", bufs=4) as rowp, \
             tc.tile_pool(name="recp", bufs=3) as recp, \
             tc.tile_pool(name="outp", bufs=3) as outp:

            # ---- persistent sbuf ----
            xt = cp.tile([128, 8, T], BF16, tag="xt")        # xT, C-part
            qT = cp.tile([128, 2, T], BF16, tag="qT")        # pair-stacked
            kT = cp.tile([128, 2, T], BF16, tag="kT")
            va = cp.tile([128, 16, HL, D + 1], BF16, tag="va")  # v + ones col
            yt = cp.tile([128, 2, T], BF16, tag="yt")
            wq16 = cp.tile([128, 8, 256], BF16, tag="wq16")
            wk16 = cp.tile([128, 8, 256], BF16, tag="wk16")
            wv16 = cp.tile([128, 8, 256], BF16, tag="wv16")
            wp16 = cp.tile([128, 2, C], BF16, tag="wp16")
            bq_sb = cp.tile([128, 2], F32, tag="bq_sb")
            bk_sb = cp.tile([128, 2], F32, tag="bk_sb")
            bv_sb = cp.tile([128, 2], F32, tag="bv_sb")
            mask_sb = cp.tile([128, 128], F32, tag="mask_sb")
            ones16 = cp.tile([1, 64], BF16, tag="ones16")

            # ---- phase 0: loads ----
            # weights first so they're resident before compute starts
            nc.gpsimd.dma_start(wq16[:], wq.rearrange("(ko p) n -> p ko n", p=128))
            nc.gpsimd.dma_start(wk16[:], wk.rearrange("(ko p) n -> p ko n", p=128))
            nc.gpsimd.dma_start(wv16[:], wv.rearrange("(ko p) n -> p ko n", p=128))
            nc.sync.dma_start(bq_sb[:], bq[:])
            nc.sync.dma_start(bk_sb[:], bk[:])
            nc.sync.dma_start(bv_sb[:], bv[:])
            nc.sync.dma_start(mask_sb[:], mask[:])
            nc.vector.memset(ones16[:], 1.0)
            nc.vector.memset(va[:, :, :, D:D + 1], 1.0)

            # x -> bf16 -> xT: plain load (HWDGE, fast) -> DVE cast ->
            # plain store -> tall xbar transposes, pipelined per 512-row
            # quarter so the first T-chunk of xT is available quickly.
            x16d = dp.tile([T, C], BF16)
            for r in range(4):
                rs = slice(512 * r, 512 * (r + 1))
                xst = ptp.tile([128, 4, C], F32, tag="xst")
                nc.sync.dma_start(xst[:], x[rs, :].rearrange(
                    "(a p) c -> p a c", p=128))
                x16s = ptp.tile([128, 4, C], BF16, tag="x16s")
                nc.vector.tensor_copy(x16s[:], xst[:])
                nc.scalar.dma_start(x16d[rs, :].rearrange(
                    "(a p) c -> p a c", p=128), x16s[:])
                for kt_i in range(8):
                    eng = nc.sync if kt_i % 2 == 0 else nc.scalar
                    eng.dma_start_transpose(
                        xt[:, kt_i, rs], x16d[rs, 128 * kt_i:128 * (kt_i + 1)])
            nc.gpsimd.dma_start(wp16[:], wp.rearrange("(ko p) n -> p ko n", p=128))

            def qkv_chunk(j):
                qs = slice(512 * j, 512 * (j + 1))
                for (w16, bias_sb, dst) in ((wq16, bq_sb, qT),
                                            (wk16, bk_sb, kT)):
                    for p in range(2):
                        psq = psp.tile([128, 512], F32, tag="ps")
                        for kt_i in range(8):
                            nc.tensor.matmul(
                                psq[:],
                                w16[:, kt_i, 128 * p:128 * (p + 1)],
                                xt[:, kt_i, qs],
                                start=(kt_i == 0), stop=(kt_i == 7))
                        nc.vector.tensor_scalar_add(
                            dst[:, p, qs], psq[:], bias_sb[:, p:p + 1])
                for t in range(4 * j, 4 * (j + 1)):
                    psv = psp.tile([128, 512], F32, tag="ps")
                    for kt_i in range(8):
                        nc.tensor.matmul(
                            psv[:, :256],
                            xt[:, kt_i, 128 * t:128 * (t + 1)],
                            wv16[:, kt_i, :],
                            start=(kt_i == 0), stop=(kt_i == 7))
                    nc.any.tensor_copy(
                        out=va[:, t, :, 0:D],
                        in_=psv[:, :256].rearrange("p (h d) -> p h d", h=HL))

            def attn_chunk(h, j):
                p, hh = h // 2, h % 2
                pr = 64 * hh
                pv = pvp.tile([65, 512], F32, tag="pv")
                nkb = 4 * j + 4
                for kb in range(nkb):
                    off = 128 * (kb - 4 * j) if kb >= 4 * j else 0
                    sps = psp.tile([128, 512], F32, tag="ps")
                    nc.tensor.matmul(
                        sps[:, off:512],
                        kT[pr:pr + 64, p, 128 * kb:128 * (kb + 1)],
                        qT[pr:pr + 64, p, 512 * j + off:512 * (j + 1)],
                        start=True, stop=True)
                    if kb >= 4 * j:
                        nc.vector.tensor_tensor(
                            sps[:, off:off + 128],
                            sps[:, off:off + 128], mask_sb[:], add)
                    ptt = ptp.tile([128, 512], BF16, tag="pt")
                    nc.scalar.activation(ptt[:, off:512], sps[:, off:512],
                                         Exp, scale=SCALE)
                    nc.tensor.matmul(
                        pv[:, off:512],
                        va[:, kb, h, :],
                        ptt[:, off:512],
                        start=(kb == 0), stop=(kb == nkb - 1))
                # normalize: yT = y' * (1/rowsum) broadcast over d
                row16 = rowp.tile([1, 512], BF16, tag="row")
                nc.vector.tensor_copy(row16[:], pv[64:65, :])
                bc = bcp.tile([64, 512], F32, tag="bc")
                nc.tensor.matmul(bc[:], ones16[:], row16[:],
                                 start=True, stop=True)
                rec = recp.tile([64, 512], F32, tag="rec")
                nc.vector.reciprocal_approx_fast(rec[:], bc[:])
                nc.vector.tensor_tensor(
                    yt[pr:pr + 64, p, 512 * j:512 * (j + 1)],
                    pv[0:64, :], rec[:], mult)
                # v bias, fused per chunk region (in-place on yT)
                nc.vector.tensor_scalar_add(
                    yt[pr:pr + 64, p, 512 * j:512 * (j + 1)],
                    yt[pr:pr + 64, p, 512 * j:512 * (j + 1)],
                    bv_sb[pr:pr + 64, p:p + 1])

            def proj_tile(t):
                osb = outp.tile([128, C], F32, tag="osb")
                for cc in range(2):
                    ops = psp.tile([128, 512], F32, tag="ps")
                    for u in range(2):
                        nc.tensor.matmul(
                            ops[:],
                            yt[:, u, 128 * t:128 * (t + 1)],
                            wp16[:, u, 512 * cc:512 * (cc + 1)],
                            start=(u == 0), stop=(u == 1))
                    nc.any.tensor_copy(out=osb[:, 512 * cc:512 * (cc + 1)],
                                       in_=ops[:])
                nc.sync.dma_start(out[128 * t:128 * (t + 1), :], osb[:])

            # ---- interleaved pipeline over T-chunks ----
            for j in range(4):
                qkv_chunk(j)
                for h in range(HL):
                    attn_chunk(h, j)
                if j >= 1:
                    for t in range(4 * (j - 1), 4 * j):
                        proj_tile(t)
            for t in range(12, 16):
                proj_tile(t)

    nc.compile()
    return nc


def make_fn(nc):
    """Sharded 8-core jit callable for the compiled Bass program."""
    import jax
    from jax.sharding import Mesh, PartitionSpec
    from jax.experimental.shard_map import shard_map

    install_neuronx_cc_hook()
    in_names, out_names, out_avals, zero_outs = [], [], [], []
    pname = nc.partition_id_tensor.name if nc.partition_id_tensor else None
    for alloc in nc.m.functions[0].allocations:
        if not isinstance(alloc, mybir.MemoryLocationSet):
            continue
        name = alloc.memorylocations[0].name
        if alloc.kind == "ExternalInput":
            if name != pname:
                in_names.append(name)
        elif alloc.kind == "ExternalOutput":
            out_names.append(name)
            shape = tuple(alloc.tensor_shape)
            dtype = mybir.dt.np(alloc.dtype)
            out_avals.append(jax.core.ShapedArray(shape, dtype))
            zero_outs.append(np.zeros(shape, dtype))
    n_params = len(in_names)
    all_names = list(in_names) + out_names
    if pname is not None:
        all_names.append(pname)

    def _body(*args):
        operands = list(args)
        if pname is not None:
            operands.append(partition_id_tensor())
        outs = _bass_exec_p.bind(
            *operands, out_avals=tuple(out_avals), in_names=tuple(all_names),
            out_names=tuple(out_names), lowering_input_output_aliases=(),
            sim_require_finite=True, sim_require_nnan=True, nc=nc)
        return tuple(outs)

    devices = jax.devices()[:N_CORES]
    mesh = Mesh(np.asarray(devices), ("core",))
    n_out = len(out_names)
    fn = jax.jit(
        shard_map(_body, mesh=mesh,
                  in_specs=(PartitionSpec("core"),) * (n_params + n_out),
                  out_specs=(PartitionSpec("core"),) * n_out,
                  check_rep=False),
        keep_unused=True)
    return fn, in_names, out_names, zero_outs


def shard_inputs(x, w_attn, b_attn, w_proj, b_proj):
    """Build the per-core input maps (core = 4*batch + head_group)."""
    x = np.asarray(x, dtype=np.float32)
    w_attn = np.asarray(w_attn, dtype=np.float32)
    b_attn = np.asarray(b_attn, dtype=np.float32)
    w_proj = np.asarray(w_proj, dtype=np.float32)
    mask = np.where(np.arange(128)[None, :] >= np.arange(128)[:, None],
                    np.float32(0.0), np.float32(NEG))
    in_maps = []
    for core in range(N_CORES):
        b, g = divmod(core, G)
        cs = slice(256 * g, 256 * (g + 1))
        bcol = lambda v: np.ascontiguousarray(v.reshape(2, 128).T)
        in_maps.append({
            "x": np.ascontiguousarray(x[b]),
            "wq": np.ascontiguousarray(w_attn[:, cs]),
            "wk": np.ascontiguousarray(w_attn[:, 1024:][:, cs]),
            "wv": np.ascontiguousarray(w_attn[:, 2048:][:, cs]),
            "bq": bcol(b_attn[cs]),
            "bk": bcol(b_attn[1024:][cs]),
            "bv": bcol(b_attn[2048:][cs]),
            "wp": np.ascontiguousarray(w_proj[cs, :]),
            "mask": mask,
        })
    return in_maps


_cache = {}
_lock = threading.Lock()


def _get_compiled():
    with _lock:
        if "fn" not in _cache:
            nc = build_nc()
            fn, in_names, out_names, zero_outs = make_fn(nc)
            _cache.update(fn=fn, nc=nc, in_names=in_names,
                          out_names=out_names, zero_outs=zero_outs)
    return _cache


def run_cores(in_maps):
    """Execute the 8-core SPMD program, return per-core output dicts."""
    import jax

    cc = _get_compiled()
    concat_in = [np.concatenate([m[k] for m in in_maps], axis=0)
                 for k in cc["in_names"]]
    concat_zeros = [np.zeros((N_CORES * z.shape[0], *z.shape[1:]), z.dtype)
                    for z in cc["zero_outs"]]
    outs = cc["fn"](*[jax.device_put(v) for v in concat_in],
                    *[jax.device_put(z) for z in concat_zeros])
    res = []
    for c in range(N_CORES):
        res.append({name: np.asarray(outs[i]).reshape(
            N_CORES, *cc["zero_outs"][i].shape)[c]
            for i, name in enumerate(cc["out_names"])})
    return res


def kernel(x, w_attn, b_attn, w_proj, b_proj):
    in_maps = shard_inputs(x, w_attn, b_attn, w_proj, b_proj)
    res = run_cores(in_maps)
    b_proj = np.asarray(b_proj, dtype=np.float32)
    out = np.empty((B, T, C), dtype=np.float32)
    for b in range(B):
        acc = res[4 * b]["out"].astype(np.float32).copy()
        for g in range(1, G):
            acc += res[4 * b + g]["out"]
        out[b] = acc + b_proj
    return out


# revision 5
# speedup vs baseline: 1.0550x; 1.0550x over previous
"""Causal self-attention (B=2, T=2048, C=1024, H=16) on 8 TRN2 NeuronCores.

Sharding: data-parallel over batch (2 groups of 4 cores) x tensor-parallel
over heads (4 heads per core). Each core computes qkv for its 4 heads,
causal flash-style attention, and a partial output projection
(y_heads @ w_proj rows). Host sums the 4 partial projections per batch and
adds b_proj.

Per-core device pipeline (all matmuls bf16, fp32 accumulation):
  x [2048,1024] --cast-dma--> bf16 --xbar-transpose--> xT [C-part, T]
  qT/kT = w.T @ xT  (head-pair tiles [128, T], 2 heads stacked)
  v     = xT.T @ wv ([T-part, 4 heads x 64], +ones column for rowsums)
  S^T[k,q] = kT.T@qT per 128k x 512q block (causal blocks only,
             lower-triangle via restricted N + additive mask on diagonal)
  P = exp(S^T/32)  (ACT, PSUM->SBUF bf16; no max-subtraction: scores are O(1))
  y'^T[d,q] (+rowsum row) = v_aug.T @ P  (PSUM accum over k blocks)
  yT = y'^T * (1/rowsum) via rank-1 broadcast matmul + reciprocal_approx
  outT tile = yT.T @ wp  -> out [T-part, C] partial, DMA to DRAM
"""
import threading

import numpy as np

import concourse.bass as bass
import concourse.tile as tile
from concourse import bacc, mybir
from concourse.bass2jax import (
    _bass_exec_p,
    install_neuronx_cc_hook,
    partition_id_tensor,
)

N_CORES = 8
B, T, C, H = 2, 2048, 1024, 16
D = C // H            # 64
HL = 4                # heads per core
G = H // HL           # 4 head groups
SCALE = 1.0 / 32.0    # 1/sqrt(C)
F32 = mybir.dt.float32
BF16 = mybir.dt.bfloat16
NEG = -1e9


def build_nc():
    nc = bacc.Bacc("TRN2", target_bir_lowering=False, debug=False,
                   num_devices=N_CORES)
    x = nc.dram_tensor("x", [T, C], F32, kind="ExternalInput").ap()
    wq = nc.dram_tensor("wq", [C, 256], F32, kind="ExternalInput").ap()
    wk = nc.dram_tensor("wk", [C, 256], F32, kind="ExternalInput").ap()
    wv = nc.dram_tensor("wv", [C, 256], F32, kind="ExternalInput").ap()
    bq = nc.dram_tensor("bq", [128, 2], F32, kind="ExternalInput").ap()
    bk = nc.dram_tensor("bk", [128, 2], F32, kind="ExternalInput").ap()
    bv = nc.dram_tensor("bv", [128, 2], F32, kind="ExternalInput").ap()
    wp = nc.dram_tensor("wp", [256, C], F32, kind="ExternalInput").ap()
    mask = nc.dram_tensor("mask", [128, 128], F32, kind="ExternalInput").ap()
    out = nc.dram_tensor("out", [T, C], F32, kind="ExternalOutput").ap()

    Exp = mybir.ActivationFunctionType.Exp
    add = mybir.AluOpType.add
    mult = mybir.AluOpType.mult

    with tile.TileContext(nc) as tc:
        with tc.tile_pool(name="dram", bufs=1, space="DRAM") as dp, \
             tc.tile_pool(name="const", bufs=1) as cp, \
             tc.tile_pool(name="ps", bufs=3, space="PSUM") as psp, \
             tc.tile_pool(name="pvps", bufs=2, space="PSUM") as pvp, \
             tc.tile_pool(name="bcps", bufs=2, space="PSUM") as bcp, \
             tc.tile_pool(name="ptp", bufs=4) as ptp, \
             tc.tile_pool(name="xprep", bufs=2) as xpp, \
             tc.tile_pool(name="rowp", bufs=4) as rowp, \
             tc.tile_pool(name="recp", bufs=3) as recp, \
             tc.tile_pool(name="outp", bufs=3) as outp:

            # ---- persistent sbuf ----
            xt = cp.tile([128, 8, T], BF16, tag="xt")        # xT, C-part
            qT = cp.tile([128, 2, T], BF16, tag="qT")        # pair-stacked
            kT = cp.tile([128, 2, T], BF16, tag="kT")
            va = cp.tile([128, 16, HL, D + 1], BF16, tag="va")  # v + ones col
            yt = cp.tile([128, 2, T], BF16, tag="yt")
            wq16 = cp.tile([128, 8, 256], BF16, tag="wq16")
            wk16 = cp.tile([128, 8, 256], BF16, tag="wk16")
            wv16 = cp.tile([128, 8, 256], BF16, tag="wv16")
            wp16 = cp.tile([128, 2, C], BF16, tag="wp16")
            bq_sb = cp.tile([128, 2], F32, tag="bq_sb")
            bk_sb = cp.tile([128, 2], F32, tag="bk_sb")
            bv_sb = cp.tile([128, 2], F32, tag="bv_sb")
            mask_sb = cp.tile([128, 128], F32, tag="mask_sb")
            ones16 = cp.tile([1, 64], BF16, tag="ones16")

            # ---- phase 0: loads ----
            # weights first so they're resident before compute starts
            nc.gpsimd.dma_start(wq16[:], wq.rearrange("(ko p) n -> p ko n", p=128))
            nc.gpsimd.dma_start(wk16[:], wk.rearrange("(ko p) n -> p ko n", p=128))
            nc.gpsimd.dma_start(wv16[:], wv.rearrange("(ko p) n -> p ko n", p=128))
            nc.sync.dma_start(bq_sb[:], bq[:])
            nc.sync.dma_start(bk_sb[:], bk[:])
            nc.sync.dma_start(bv_sb[:], bv[:])
            nc.sync.dma_start(mask_sb[:], mask[:])
            nc.vector.memset(ones16[:], 1.0)
            nc.vector.memset(va[:, :, :, D:D + 1], 1.0)

            # x -> bf16 -> xT: plain load (HWDGE, fast) -> DVE cast ->
            # plain store -> tall xbar transposes, pipelined per 512-row
            # quarter so the first T-chunk of xT is available quickly.
            x16d = dp.tile([T, C], BF16)
            for r in range(4):
                rs = slice(512 * r, 512 * (r + 1))
                xst = xpp.tile([128, 4, C], F32, tag="xst")
                nc.gpsimd.dma_start(xst[:], x[rs, :].rearrange(
                    "(a p) c -> p a c", p=128))
                x16s = xpp.tile([128, 4, C], BF16, tag="x16s")
                nc.vector.tensor_copy(x16s[:], xst[:])
                nc.sync.dma_start(x16d[rs, :].rearrange(
                    "(a p) c -> p a c", p=128), x16s[:])
                for kt_i in range(8):
                    nc.sync.dma_start_transpose(
                        xt[:, kt_i, rs], x16d[rs, 128 * kt_i:128 * (kt_i + 1)])
            nc.gpsimd.dma_start(wp16[:], wp.rearrange("(ko p) n -> p ko n", p=128))

            def qkv_chunk(j):
                qs = slice(512 * j, 512 * (j + 1))
                for (w16, bias_sb, dst) in ((wq16, bq_sb, qT),
                                            (wk16, bk_sb, kT)):
                    for p in range(2):
                        psq = psp.tile([128, 512], F32, tag="ps")
                        for kt_i in range(8):
                            nc.tensor.matmul(
                                psq[:],
                                w16[:, kt_i, 128 * p:128 * (p + 1)],
                                xt[:, kt_i, qs],
                                start=(kt_i == 0), stop=(kt_i == 7))
                        nc.vector.tensor_scalar_add(
                            dst[:, p, qs], psq[:], bias_sb[:, p:p + 1])
                for t in range(4 * j, 4 * (j + 1)):
                    psv = psp.tile([128, 512], F32, tag="ps")
                    for kt_i in range(8):
                        nc.tensor.matmul(
                            psv[:, :256],
                            xt[:, kt_i, 128 * t:128 * (t + 1)],
                            wv16[:, kt_i, :],
                            start=(kt_i == 0), stop=(kt_i == 7))
                    nc.vector.tensor_copy(
                        out=va[:, t, :, 0:D],
                        in_=psv[:, :256].rearrange("p (h d) -> p h d", h=HL))

            def attn_chunk(h, j):
                p, hh = h // 2, h % 2
                pr = 64 * hh
                pv = pvp.tile([65, 512], F32, tag="pv")
                nkb = 4 * j + 4
                for kb in range(nkb):
                    off = 128 * (kb - 4 * j) if kb >= 4 * j else 0
                    sps = psp.tile([128, 512], F32, tag="ps")
                    nc.tensor.matmul(
                        sps[:, off:512],
                        kT[pr:pr + 64, p, 128 * kb:128 * (kb + 1)],
                        qT[pr:pr + 64, p, 512 * j + off:512 * (j + 1)],
                        start=True, stop=True)
                    if kb >= 4 * j:
                        nc.vector.tensor_tensor(
                            sps[:, off:off + 128],
                            sps[:, off:off + 128], mask_sb[:], add)
                    ptt = ptp.tile([128, 512], BF16, tag="pt")
                    nc.scalar.activation(ptt[:, off:512], sps[:, off:512],
                                         Exp, scale=SCALE)
                    nc.tensor.matmul(
                        pv[:, off:512],
                        va[:, kb, h, :],
                        ptt[:, off:512],
                        start=(kb == 0), stop=(kb == nkb - 1))
                # normalize: yT = y' * (1/rowsum) broadcast over d
                row16 = rowp.tile([1, 512], BF16, tag="row")
                nc.vector.tensor_copy(row16[:], pv[64:65, :])
                bc = bcp.tile([64, 512], F32, tag="bc")
                nc.tensor.matmul(bc[:], ones16[:], row16[:],
                                 start=True, stop=True)
                rec = recp.tile([64, 512], F32, tag="rec")
                nc.vector.reciprocal_approx_fast(rec[:], bc[:])
                nc.vector.tensor_tensor(
                    yt[pr:pr + 64, p, 512 * j:512 * (j + 1)],
                    pv[0:64, :], rec[:], mult)
                # v bias, fused per chunk region (in-place on yT)
                nc.vector.tensor_scalar_add(
                    yt[pr:pr + 64, p, 512 * j:512 * (j + 1)],
                    yt[pr:pr + 64, p, 512 * j:512 * (j + 1)],
                    bv_sb[pr:pr + 64, p:p + 1])

            def proj_tile(t):
                osb = outp.tile([128, C], F32, tag="osb")
                for cc in range(2):
                    ops = psp.tile([128, 512], F32, tag="ps")
                    for u in range(2):
                        nc.tensor.matmul(
                            ops[:],
                            yt[:, u, 128 * t:128 * (t + 1)],
                            wp16[:, u, 512 * cc:512 * (cc + 1)],
                            start=(u == 0), stop=(u == 1))
                    nc.any.tensor_copy(out=osb[:, 512 * cc:512 * (cc + 1)],
                                       in_=ops[:])
                nc.gpsimd.dma_start(out[128 * t:128 * (t + 1), :], osb[:])

            # ---- interleaved pipeline over T-chunks ----
            for j in range(4):
                qkv_chunk(j)
                for h in range(HL):
                    attn_chunk(h, j)
                if j >= 1:
                    for t in range(4 * (j - 1), 4 * j):
                        proj_tile(t)
            for t in range(12, 16):
                proj_tile(t)

    nc.compile()
    return nc


def make_fn(nc):
    """Sharded 8-core jit callable for the compiled Bass program."""
    import jax
    from jax.sharding import Mesh, PartitionSpec
    from jax.experimental.shard_map import shard_map

    install_neuronx_cc_hook()
    in_names, out_names, out_avals, zero_outs = [], [], [], []
    pname = nc.partition_id_tensor.name if nc.partition_id_tensor else None
    for alloc in nc.m.functions[0].allocations:
        if not isinstance(alloc, mybir.MemoryLocationSet):
            continue
        name = alloc.memorylocations[0].name
        if alloc.kind == "ExternalInput":
            if name != pname:
                in_names.append(name)
        elif alloc.kind == "ExternalOutput":
            out_names.append(name)
            shape = tuple(alloc.tensor_shape)
            dtype = mybir.dt.np(alloc.dtype)
            out_avals.append(jax.core.ShapedArray(shape, dtype))
            zero_outs.append(np.zeros(shape, dtype))
    n_params = len(in_names)
    all_names = list(in_names) + out_names
    if pname is not None:
        all_names.append(pname)

    def _body(*args):
        operands = list(args)
        if pname is not None:
            operands.append(partition_id_tensor())
        outs = _bass_exec_p.bind(
            *operands, out_avals=tuple(out_avals), in_names=tuple(all_names),
            out_names=tuple(out_names), lowering_input_output_aliases=(),
            sim_require_finite=True, sim_require_nnan=True, nc=nc)
        return tuple(outs)

    devices = jax.devices()[:N_CORES]
    mesh = Mesh(np.asarray(devices), ("core",))
    n_out = len(out_names)
    fn = jax.jit(
        shard_map(_body, mesh=mesh,
                  in_specs=(PartitionSpec("core"),) * (n_params + n_out),
                  out_specs=(PartitionSpec("core"),) * n_out,
                  check_rep=False),
        keep_unused=True)
    return fn, in_names, out_names, zero_outs


def shard_inputs(x, w_attn, b_attn, w_proj, b_proj):
    """Build the per-core input maps (core = 4*batch + head_group)."""
    x = np.asarray(x, dtype=np.float32)
    w_attn = np.asarray(w_attn, dtype=np.float32)
    b_attn = np.asarray(b_attn, dtype=np.float32)
    w_proj = np.asarray(w_proj, dtype=np.float32)
    mask = np.where(np.arange(128)[None, :] >= np.arange(128)[:, None],
                    np.float32(0.0), np.float32(NEG))
    in_maps = []
    for core in range(N_CORES):
        b, g = divmod(core, G)
        cs = slice(256 * g, 256 * (g + 1))
        bcol = lambda v: np.ascontiguousarray(v.reshape(2, 128).T)
        in_maps.append({
            "x": np.ascontiguousarray(x[b]),
            "wq": np.ascontiguousarray(w_attn[:, cs]),
            "wk": np.ascontiguousarray(w_attn[:, 1024:][:, cs]),
            "wv": np.ascontiguousarray(w_attn[:, 2048:][:, cs]),
            "bq": bcol(b_attn[cs]),
            "bk": bcol(b_attn[1024:][cs]),
            "bv": bcol(b_attn[2048:][cs]),
            "wp": np.ascontiguousarray(w_proj[cs, :]),
            "mask": mask,
        })
    return in_maps


_cache = {}
_lock = threading.Lock()


def _get_compiled():
    with _lock:
        if "fn" not in _cache:
            nc = build_nc()
            fn, in_names, out_names, zero_outs = make_fn(nc)
            _cache.update(fn=fn, nc=nc, in_names=in_names,
                          out_names=out_names, zero_outs=zero_outs)
    return _cache


def run_cores(in_maps):
    """Execute the 8-core SPMD program, return per-core output dicts."""
    import jax

    cc = _get_compiled()
    concat_in = [np.concatenate([m[k] for m in in_maps], axis=0)
                 for k in cc["in_names"]]
    concat_zeros = [np.zeros((N_CORES * z.shape[0], *z.shape[1:]), z.dtype)
                    for z in cc["zero_outs"]]
    outs = cc["fn"](*[jax.device_put(v) for v in concat_in],
                    *[jax.device_put(z) for z in concat_zeros])
    res = []
    for c in range(N_CORES):
        res.append({name: np.asarray(outs[i]).reshape(
            N_CORES, *cc["zero_outs"][i].shape)[c]
            for i, name in enumerate(cc["out_names"])})
    return res


def kernel(x, w_attn, b_attn, w_proj, b_proj):
    in_maps = shard_inputs(x, w_attn, b_attn, w_proj, b_proj)
    res = run_cores(in_maps)
    b_proj = np.asarray(b_proj, dtype=np.float32)
    out = np.empty((B, T, C), dtype=np.float32)
    for b in range(B):
        acc = res[4 * b]["out"].astype(np.float32).copy()
        for g in range(1, G):
            acc += res[4 * b + g]["out"]
        out[b] = acc + b_proj
    return out
